# revision 12
# baseline (speedup 1.0000x reference)
"""Trainium2 Bass kernel for nn_CLOSEgaps (hypergraph attention conv), 8 NeuronCores.

Algorithmic collapse (validated vs reference + numpy mirror):
  per-node 12-vector table:
    s_n[h] = x @ (W_conv_h @ att[h,:128]);  p[h,c] = x @ (W_conv_h @ W_out)
    wav[h] = (W_attr + b_attr/16) @ (W_conv_h @ att[h,128:])
  per-pair: e = exp(lrelu(s_n + s_e)), Z, T, qq, v (2 f32).
  zacc[n,c] = sum of v over pairs at n;  z = Dv*zacc + C
  out[m,c] = sum_{n in e_m} z[n,c]  (fp8 incidence matmul on PE)

v4: the HBM DMA-gather (16K descriptors, ~100us of Pool desc-gen) is
replaced by an all-SBUF channel-major pipeline:
  - table built TRANSPOSED ([12ch, 1024 nodes] per core) directly by PE,
    AllGathered as [128=(rank,ch), 1024], then replicated to all 8
    Q7 groups via one-hot partition-mix matmuls -> Trep [128, 8192] f32.
  - ONE GpSimd ap_gather (per-16-partition-group indices = global node
    ids) expands Trep to per-pair columns Gcol [128, 2048].
  - attention runs column-major: free dim = (k=16, edge=128) per group,
    per-edge softmax reductions are stride-128 free reduces; cross-
    channel moves are one-hot stationary matmuls (Msn/Me).
  - v is transposed back to row-major pair planes (16 PE transposes) and
    fed to the unchanged local_scatter permutation network -> zacc ->
    AllReduce -> z -> fp8 incidence matmul -> out.
"""
import sys

sys.path.insert(0, "/opt/trn_rl_repo")

import numpy as np

N = 8192
M = 8192
F_IN = 512
EMB = 256
CD = 128
H = 3
DEG = 16
NCORES = 8
NL = N // NCORES      # nodes per core
ML = M // NCORES      # edges per core
EL = ML * DEG         # pairs per core = 16384
LANES = 16            # node-slot lanes in the permuted layout

_CACHE = {}


def _build_program():
    import concourse.bass as bass
    import concourse.bacc as bacc
    import concourse.tile as tile
    from concourse import mybir

    f32 = mybir.dt.float32
    i16 = mybir.dt.int16
    bf16 = mybir.dt.bfloat16

    nc = bacc.Bacc("TRN2", target_bir_lowering=False, debug=False, num_devices=NCORES)

    # ---- per-core external inputs (host pre-laid-out) ----
    wenc_in = nc.dram_tensor("wenc_in", [128, 4 * 256], bf16, kind="ExternalInput").ap()
    ift_in = nc.dram_tensor("ift_in", [128, 4 * 1024], bf16, kind="ExternalInput").ap()
    wattr_in = nc.dram_tensor("wattr_in", [128, 2 * 1024], bf16, kind="ExternalInput").ap()
    # i16 blob: colidx [128] | r1 [128] | r2 [128] | r3 [1024]
    i16b_in = nc.dram_tensor("i16b_in", [128, 1424], i16, kind="ExternalInput").ap()
    # bf16 blob: wconv [768] | pproj [36] | brep [8*128] | msn [128] | me [128] | ident [128]
    BFW = 768 + 36 + 8 * 128 + 128 + 128 + 128
    bfb_in = nc.dram_tensor("bfb_in", [128, BFW], bf16, kind="ExternalInput").ap()
    # f32 blob: benc [2] | dv [64] | cc [2]
    f32b_in = nc.dram_tensor("f32b_in", [128, 68], f32, kind="ExternalInput").ap()
    inc_in = nc.dram_tensor("inc_in", [128, 8 * 64 * 128], mybir.dt.float8e4, kind="ExternalInput").ap()
    out_dram = nc.dram_tensor("out", [128, 16], f32, kind="ExternalOutput").ap()
    dbg_dram = nc.dram_tensor("dbg", [128, 2560], f32, kind="ExternalOutput").ap()

    with tile.TileContext(nc) as tc:
        with (
            tc.tile_pool(name="sbuf", bufs=1) as sb,
            tc.tile_pool(name="big", bufs=1) as bigp,
            tc.tile_pool(name="psum", bufs=2, space="PSUM") as ps,
            tc.tile_pool(name="psum1", bufs=1, space="PSUM") as ps1,
            tc.tile_pool(name="psbig", bufs=1, space="PSUM") as psb,
            tc.tile_pool(name="dram", bufs=1, space="DRAM") as dram,
        ):
            # ------- input loads (priority order) -------
            wenc_t = sb.tile([128, 4, 256], bf16)
            nc.sync.dma_start(out=wenc_t[:].rearrange("p a b -> p (a b)"), in_=wenc_in[:])
            ift_t = sb.tile([128, 4, 1024], bf16)
            nc.sync.dma_start(out=ift_t[:].rearrange("p a b -> p (a b)"), in_=ift_in[:])
            wattr_t = sb.tile([128, 2, 1024], bf16)
            nc.sync.dma_start(out=wattr_t[:].rearrange("p a b -> p (a b)"), in_=wattr_in[:])
            i16b_t = sb.tile([128, 1424], i16)
            nc.sync.dma_start(out=i16b_t[:], in_=i16b_in[:])
            bfb_t = sb.tile([128, BFW], bf16)
            nc.sync.dma_start(out=bfb_t[:], in_=bfb_in[:])
            f32b_t = sb.tile([128, 68], f32)
            nc.sync.dma_start(out=f32b_t[:], in_=f32b_in[:])

            colidx = i16b_t[:, 0:128]
            r1_v = i16b_t[:, 128:256]
            r2_v = i16b_t[:, 256:384]
            r3_v = i16b_t[:, 384:1408]
            zidx_v = i16b_t[:, 1408:1424]
            wconv_v = bfb_t[:, 0:768].rearrange("p (a b) -> p a b", a=3)
            pproj_v = bfb_t[:, 768:804].rearrange("p (a b) -> p a b", a=3)
            brep_v = bfb_t[:, 804:1828].rearrange("p (a b) -> p a b", a=8)
            msn_v = bfb_t[:, 1828:1956]
            me_v = bfb_t[:, 1956:2084]
            ident_v = bfb_t[:, 2084:2212]
            benc_v = f32b_t[:, 0:2]
            dv_v = f32b_t[:, 2:66]
            cc_v = f32b_t[:, 66:68]

            # dummy ap_gather: triggers the ucode library IRAM load early so
            # the real gather doesn't stall ~57us on LOAD_LIB
            scrapG = sb.tile([128, 16], f32)
            nc.gpsimd.ap_gather(
                out_ap=scrapG[:], in_ap=f32b_t[:, 0:64], idxs_ap=zidx_v,
                channels=128, num_elems=64, d=1, num_idxs=16,
            )

            # ------- P1: xT[e, n] = relu(W_enc.T @ IF.T + b_enc), emb-major --
            xT_t = sb.tile([128, 2, 1024], bf16)
            for eh in range(2):
                for nh in range(2):
                    px = ps.tile([128, 512], f32, tag="px")
                    for kc in range(4):
                        nc.tensor.matmul(
                            out=px[:],
                            lhsT=wenc_t[:, kc, eh * 128 : (eh + 1) * 128],
                            rhs=ift_t[:, kc, nh * 512 : (nh + 1) * 512],
                            start=(kc == 0),
                            stop=(kc == 3),
                        )
                    nc.scalar.activation(
                        out=xT_t[:, eh, nh * 512 : (nh + 1) * 512],
                        in_=px[:],
                        func=mybir.ActivationFunctionType.Relu,
                        bias=benc_v[:, eh : eh + 1],
                    )

            # ------- P2: UV = W_conv.T-chunks @ P_proj  ([256(2x128), 12]) --
            uv_t = sb.tile([128, 2, 12], bf16)
            for eh in range(2):
                pu_full = ps.tile([128, 512], f32, tag="px")
                pu = pu_full[0:128, 0:12]
                for qc in range(3):
                    nc.tensor.matmul(
                        out=pu,
                        lhsT=wconv_v[:, qc, eh * 128 : (eh + 1) * 128],
                        rhs=pproj_v[:, qc, :],
                        start=(qc == 0),
                        stop=(qc == 2),
                    )
                nc.vector.tensor_copy(uv_t[:, eh, :], pu)

            # ------- P2bT: transposed table slab tsliceT[16, 1024] ----------
            # zero-padded uv stationaries so all 12 rows accumulate in one
            # PSUM tile starting at partition 0 (no partition-offset writes)
            uvz = sb.tile([128, 4, 12], bf16)
            nc.vector.memset(uvz[:], 0.0)
            for eh in range(2):
                nc.vector.tensor_copy(uvz[:, eh, 0:9], uv_t[:, eh, 0:9])
                nc.vector.tensor_copy(uvz[:, 2 + eh, 9:12], uv_t[:, eh, 9:12])
            tsl_sb = sb.tile([16, 1024], f32)
            nc.vector.memset(tsl_sb[:], 0.0)
            for jb in range(2):
                pt16 = ps.tile([128, 512], f32, tag="px")
                for eh in range(2):
                    nc.tensor.matmul(
                        out=pt16[0:12, :],
                        lhsT=uvz[:, eh, :],
                        rhs=xT_t[:, eh, jb * 512 : (jb + 1) * 512],
                        start=(eh == 0),
                        stop=False,
                    )
                for ec in range(2):
                    nc.tensor.matmul(
                        out=pt16[0:12, :],
                        lhsT=uvz[:, 2 + ec, :],
                        rhs=wattr_t[:, ec, jb * 512 : (jb + 1) * 512],
                        start=False,
                        stop=(ec == 1),
                    )
                nc.vector.tensor_copy(tsl_sb[0:12, jb * 512 : (jb + 1) * 512], pt16[0:12, :])

            # ------- AllGather transposed table: trepg [128=(r,ch), 1024] ---
            tsliceT = dram.tile([16, 1024], f32)
            nc.sync.dma_start(out=tsliceT[:], in_=tsl_sb[:])
            trepg_d = dram.tile([128, 1024], f32)
            nc.gpsimd.collective_compute(
                "AllGather",
                mybir.AluOpType.bypass,
                replica_groups=[list(range(NCORES))],
                ins=[tsliceT.opt()],
                outs=[trepg_d.opt()],
            )
            trepg_sb = sb.tile([128, 1024], f32)
            nc.sync.dma_start(out=trepg_sb[:], in_=trepg_d[:])
            trepg_bf = sb.tile([128, 1024], bf16)
            nc.vector.tensor_copy(trepg_bf[:], trepg_sb[:])

            # ------- Trep build: [128=(g,ch), 8192 nodes] f32 ----------------
            trep = bigp.tile([128, 8, 1024], f32)
            for r in range(8):
                for jb in range(2):
                    pm = ps.tile([128, 512], f32, tag="px")
                    nc.tensor.matmul(
                        out=pm[:],
                        lhsT=brep_v[:, r, :],
                        rhs=trepg_bf[:, jb * 512 : (jb + 1) * 512],
                        start=True,
                        stop=True,
                    )
                    if jb == 0:
                        nc.vector.tensor_copy(trep[:, r, 0:512], pm[:])
                    else:
                        nc.scalar.activation(
                            out=trep[:, r, 512:1024], in_=pm[:],
                            func=mybir.ActivationFunctionType.Copy,
                        )

            # ------- THE gather: Gcol[16g+ch, t*128+j] = table[ch, node] ----
            Gcol = sb.tile([128, 2048], f32)
            nc.gpsimd.ap_gather(
                out_ap=Gcol[:],
                in_ap=trep[:].rearrange("p a b -> p (a b)"),
                idxs_ap=colidx,
                channels=128,
                num_elems=N,
                d=1,
                num_idxs=2048,
            )

            # ------- non-critical loads ----
            incs_t = bigp.tile([128, 8, 64, 128], mybir.dt.float8e4)
            nc.sync.dma_start(out=incs_t[:].rearrange("p a b m -> p (a b m)"), in_=inc_in[:])

            # ------- column-major attention --------------------------------
            # rows per 16-group: 0-2 sn, 3-8 p(h,c), 9-11 wav, 12-15 zero
            se_col = sb.tile([128, 128], f32)
            nc.vector.reduce_sum(
                out=se_col[:][:, :, None],
                in_=Gcol[:].rearrange("q (t j) -> q j t", j=128),
                axis=mybir.AxisListType.X,
            )
            se_bf = sb.tile([128, 128], bf16)
            nc.vector.tensor_copy(se_bf[:], se_col[:])
            se_ps = ps1.tile([128, 128], f32, tag="seps")
            nc.tensor.matmul(out=se_ps[:], lhsT=msn_v, rhs=se_bf[:], start=True, stop=True)

            lg = sb.tile([128, 2048], f32)
            nc.vector.tensor_tensor(
                out=lg[:].rearrange("q (t j) -> q t j", t=16),
                in0=Gcol[:].rearrange("q (t j) -> q t j", t=16),
                in1=se_ps[:][:, None, :].to_broadcast([128, 16, 128]),
                op=mybir.AluOpType.add,
            )
            lgs = sb.tile([128, 2048], f32)
            nc.vector.tensor_scalar_mul(lgs[:], lg[:], 0.2)
            nc.vector.tensor_tensor(out=lg[:], in0=lg[:], in1=lgs[:], op=mybir.AluOpType.max)
            ee_bf = sb.tile([128, 2048], bf16)
            nc.scalar.activation(out=ee_bf[:], in_=lg[:], func=mybir.ActivationFunctionType.Exp)

            zc = sb.tile([128, 128], f32)
            nc.vector.reduce_sum(
                out=zc[:][:, :, None],
                in_=ee_bf[:].rearrange("q (t j) -> q j t", j=128),
                axis=mybir.AxisListType.X,
            )
            nc.vector.tensor_scalar_add(zc[:], zc[:], 1e-16)
            zr = sb.tile([128, 128], f32)
            nc.vector.reciprocal(zr[:], zc[:])
            zr2b = sb.tile([128, 128], bf16)
            nc.vector.tensor_tensor(out=zr[:], in0=zr[:], in1=zr[:], op=mybir.AluOpType.mult)
            nc.vector.tensor_scalar_mul(zr[:], zr[:], 1.0 / DEG)
            nc.vector.tensor_copy(zr2b[:], zr[:])

            pdup = psb.tile([128, 2048], f32, tag="pdup")
            for cb in range(4):
                nc.tensor.matmul(
                    out=pdup[:, cb * 512 : (cb + 1) * 512],
                    lhsT=me_v,
                    rhs=ee_bf[:, cb * 512 : (cb + 1) * 512],
                    start=True,
                    stop=True,
                )
            zdup = ps1.tile([128, 128], f32, tag="seps")
            nc.tensor.matmul(out=zdup[:], lhsT=me_v, rhs=zr2b[:], start=True, stop=True)

            tq = sb.tile([128, 2048], f32)
            nc.vector.tensor_tensor(out=tq[:], in0=pdup[:], in1=Gcol[:], op=mybir.AluOpType.mult)
            tcol = sb.tile([128, 128], f32)
            nc.vector.reduce_sum(
                out=tcol[:][:, :, None],
                in_=tq[:].rearrange("q (t j) -> q j t", j=128),
                axis=mybir.AxisListType.X,
            )
            qq = sb.tile([128, 128], f32)
            nc.vector.tensor_tensor(out=qq[:], in0=tcol[:], in1=zdup[:], op=mybir.AluOpType.mult)

            w_bf = sb.tile([128, 2048], bf16)
            nc.vector.tensor_tensor(
                out=w_bf[:].rearrange("q (t j) -> q t j", t=16),
                in0=pdup[:].rearrange("q (t j) -> q t j", t=16),
                in1=qq[:][:, None, :].to_broadcast([128, 16, 128]),
                op=mybir.AluOpType.mult,
            )

            # ------- column -> row-major pair planes V0/V1 ------------------
            Vb = sb.tile([128, 16, 8, 2], f32)
            for t in range(16):
                ptb = ps1.tile([128, 128], bf16, tag="pt")
                nc.tensor.transpose(ptb[:], w_bf[:, t * 128 : (t + 1) * 128], ident_v)
                nc.vector.reduce_sum(
                    out=Vb[:, t, :, :],
                    in_=ptb[:].rearrange("j (g s) -> j g s", s=16)[:, :, 3:9]
                        .rearrange("j g (h c) -> j g c h", c=2),
                    axis=mybir.AxisListType.X,
                )
            V0 = sb.tile([128, 128], bf16)
            V1 = sb.tile([128, 128], bf16)
            nc.vector.tensor_copy(
                V0[:].rearrange("p (g t2) -> p g t2", t2=16),
                Vb[:, :, :, 0].rearrange("p t g -> p g t"),
            )
            nc.vector.tensor_copy(
                V1[:].rearrange("p (g t2) -> p g t2", t2=16),
                Vb[:, :, :, 1].rearrange("p t g -> p g t"),
            )

            nc.sync.dma_start(out=dbg_dram[:, 0:2048], in_=Gcol[:])
            V0f = sb.tile([128, 128], f32)
            V1f = sb.tile([128, 128], f32)
            nc.vector.tensor_copy(V0f[:], V0[:])
            nc.vector.tensor_copy(V1f[:], V1[:])
            nc.sync.dma_start(out=dbg_dram[:, 2048:2176], in_=V0f[:])
            nc.sync.dma_start(out=dbg_dram[:, 2176:2304], in_=V1f[:])

            # ------- permutation network: R1 -> T -> R2 -> T -> R3 -------
            S1a = sb.tile([128, 128], bf16)
            S1b = sb.tile([128, 128], bf16)
            S1Ta = sb.tile([128, 128], bf16)
            S1Tb = sb.tile([128, 128], bf16)
            S2a = sb.tile([128, 1024], bf16)
            S2b = sb.tile([128, 1024], bf16)
            S2Ta = sb.tile([128, 1024], bf16)
            S2Tb = sb.tile([128, 1024], bf16)
            S3a = sb.tile([128, 1024], bf16)
            S3b = sb.tile([128, 1024], bf16)
            S1 = [S1a, S1b]
            S1T = [S1Ta, S1Tb]
            S2 = [S2a, S2b]
            S2T = [S2Ta, S2Tb]
            S3 = [S3a, S3b]
            for c, V in enumerate([V0, V1]):
                nc.gpsimd.local_scatter(
                    out_ap=S1[c][:], data_ap=V[:], idxs_ap=r1_v,
                    channels=128, num_elems=128, num_idxs=128,
                )
            for c in range(2):
                pt = ps1.tile([128, 128], bf16, tag="pt")
                nc.tensor.transpose(pt[:], S1[c][:], ident_v)
                nc.vector.tensor_copy(S1T[c][:], pt[:])
            for c in range(2):
                nc.gpsimd.local_scatter(
                    out_ap=S2[c][:], data_ap=S1T[c][:], idxs_ap=r2_v,
                    channels=128, num_elems=1024, num_idxs=128,
                )
            for c in range(2):
                for sq in range(8):
                    pt = ps1.tile([128, 128], bf16, tag="pt")
                    nc.tensor.transpose(
                        pt[:], S2[c][:, 128 * sq : 128 * (sq + 1)], ident_v
                    )
                    nc.vector.tensor_copy(S2T[c][:, 128 * sq : 128 * (sq + 1)], pt[:])
            for c in range(2):
                nc.gpsimd.local_scatter(
                    out_ap=S3[c][:], data_ap=S2T[c][:], idxs_ap=r3_v,
                    channels=128, num_elems=1024, num_idxs=1024,
                )

            # ------- lane reduce: zacc[p, nr, c], node = nr*128 + p -------
            zacc = sb.tile([128, 64, 2], f32)
            for c in range(2):
                nc.vector.reduce_sum(
                    out=zacc[:, :, c][:, :, None],
                    in_=S3[c][:].rearrange("p (nr l) -> p nr l", l=LANES),
                    axis=mybir.AxisListType.X,
                )

            nc.sync.dma_start(out=dbg_dram[:, 2304:2432], in_=zacc[:].rearrange("p a b -> p (a b)"))
            # ------- AllReduce zacc -------
            ar_in = dram.tile([128, 128], f32)
            nc.sync.dma_start(out=ar_in[:], in_=zacc[:].rearrange("p a b -> p (a b)"))
            ar_out = dram.tile([128, 128], f32)
            nc.gpsimd.collective_compute(
                "AllReduce",
                mybir.AluOpType.add,
                replica_groups=[list(range(NCORES))],
                ins=[ar_in.opt()],
                outs=[ar_out.opt()],
            )
            zred = sb.tile([128, 64, 2], f32)
            nc.sync.dma_start(out=zred[:].rearrange("p a b -> p (a b)"), in_=ar_out[:])

            nc.sync.dma_start(out=dbg_dram[:, 2432:2560], in_=zred[:].rearrange("p a b -> p (a b)"))
            # ------- z = Dv*zacc + C; bf16 hi/lo split -------
            nc.vector.tensor_tensor(
                out=zred[:],
                in0=zred[:],
                in1=dv_v[:, :, None].to_broadcast([128, 64, 2]),
                op=mybir.AluOpType.mult,
            )
            nc.vector.tensor_tensor(
                out=zred[:],
                in0=zred[:],
                in1=cc_v[:, None, :].to_broadcast([128, 64, 2]),
                op=mybir.AluOpType.add,
            )
            zz4 = sb.tile([128, 64, 4], bf16)
            nc.vector.tensor_copy(zz4[:, :, 0:2], zred[:])
            zhi32 = sb.tile([128, 64, 2], f32)
            nc.vector.tensor_copy(zhi32[:], zz4[:, :, 0:2])
            nc.vector.tensor_tensor(
                out=zhi32[:], in0=zred[:], in1=zhi32[:], op=mybir.AluOpType.subtract
            )
            nc.vector.tensor_copy(zz4[:, :, 2:4], zhi32[:])

            # ------- final: out[p_e, c] = sum_n inc[n, edge] * z[n] via PE --
            out_t = sb.tile([128, 8, 2], f32)
            for j in range(8):
                po_full = ps1.tile([128, 128], f32, tag="seps")
                po = po_full[0:128, 0:4]
                for nck in range(64):
                    nc.tensor.matmul(
                        out=po,
                        lhsT=incs_t[:, j, nck, :],
                        rhs=zz4[:, nck, :],
                        start=(nck == 0),
                        stop=(nck == 63),
                    )
                nc.vector.tensor_copy(out_t[:, j, :], po_full[:, 0:2])
                nc.vector.tensor_tensor(
                    out=out_t[:, j, :], in0=out_t[:, j, :], in1=po_full[:, 2:4],
                    op=mybir.AluOpType.add,
                )
            nc.sync.dma_start(
                out=out_dram[:], in_=out_t[:].rearrange("p a b -> p (a b)")
            )

    nc.compile()
    return nc


def _host_prep(inputs):
    """Build per-core in_maps from full inputs."""
    import ml_dtypes

    bf = ml_dtypes.bfloat16
    IF = np.asarray(inputs["input_features"], np.float32)
    node_idx = np.asarray(inputs["node_idx"])
    W_enc = np.asarray(inputs["W_enc"], np.float32)
    b_enc = np.asarray(inputs["b_enc"], np.float32)
    W_attr = np.asarray(inputs["W_attr"], np.float32)
    b_attr = np.asarray(inputs["b_attr"], np.float32)
    W_conv = np.asarray(inputs["W_conv"], np.float32)
    att = np.asarray(inputs["att"], np.float32)
    b_conv = np.asarray(inputs["b_conv"], np.float32)
    W_out = np.asarray(inputs["W_out"], np.float32)
    b_out = np.asarray(inputs["b_out"], np.float32)

    nodes16 = node_idx.reshape(M, DEG).astype(np.int64)

    # weight prep
    P_proj = np.zeros((H * CD, 12), np.float32)
    for h in range(H):
        P_proj[h * CD : (h + 1) * CD, h] = att[h, :CD]
        for cc in range(2):
            P_proj[h * CD : (h + 1) * CD, 3 + h * 2 + cc] = W_out[h * CD : (h + 1) * CD, cc]
        P_proj[h * CD : (h + 1) * CD, 9 + h] = att[h, CD:]

    deg_n = np.bincount(node_idx, minlength=N)
    Dv = np.where(deg_n > 0, 1.0 / np.maximum(deg_n, 1), 0.0).astype(np.float32)
    C = (b_conv @ W_out + b_out / DEG).astype(np.float32)

    wenc_l = W_enc.reshape(4, 128, EMB).transpose(1, 0, 2).reshape(128, -1).astype(bf).copy()
    benc_l = b_enc.reshape(2, 128).T.astype(np.float32)
    wconv_l = W_conv.T.reshape(3, 128, EMB).transpose(1, 0, 2).reshape(128, -1)
    pproj_l = P_proj.reshape(3, 128, 12).transpose(1, 0, 2).reshape(128, -1)
    cc_l = np.tile(C[None, :], (128, 1))
    dv_l = Dv.reshape(64, 128).T

    # one-hot partition-mix matrices
    brep = np.zeros((128, 8, 128), np.float32)
    msn = np.zeros((128, 128), np.float32)
    me = np.zeros((128, 128), np.float32)
    for g in range(8):
        for ch in range(12):
            for r_ in [g]:
                pass
    for r_ in range(8):
        for g in range(8):
            for ch in range(12):
                brep[16 * r_ + ch, r_, 16 * g + ch] = 1.0
    for g in range(8):
        for h in range(3):
            msn[16 * g + 9 + h, 16 * g + h] = 1.0
            for ci in range(2):
                me[16 * g + h, 16 * g + 3 + 2 * h + ci] = 1.0
    ident_np = np.eye(128, dtype=np.float32)
    bfb_l = np.concatenate(
        [wconv_l, pproj_l, brep.reshape(128, -1), msn, me, ident_np], axis=1
    ).astype(bf).copy()
    f32b_l = np.concatenate([benc_l, dv_l, cc_l], axis=1).astype(np.float32).copy()

    # pair order: i = blk*128 + p, blk = 16g+t; edge = g*128+p, k=t
    i = np.arange(EL)
    p_of = i % 128
    blk = i // 128
    jj = blk // 16
    k_of = blk % 16

    ll = np.arange(2048)
    jj2 = ll % 128
    tt = ll // 128

    in_maps = []
    for c in range(NCORES):
        nsl = slice(c * NL, (c + 1) * NL)
        esl = slice(c * ML, (c + 1) * ML)
        ift_l = (
            IF[nsl].T.reshape(4, 128, 1024).transpose(1, 0, 2).reshape(128, -1).astype(bf).copy()
        )
        wattr_l = (
            (W_attr[nsl] + b_attr[None, :] / DEG)
            .T.reshape(2, 128, 1024)
            .transpose(1, 0, 2)
            .reshape(128, -1)
            .astype(bf)
            .copy()
        )
        nsub = nodes16[esl]                       # [1024, 16]
        node_of = nsub[jj * 128 + p_of, k_of]     # node id per pair i

        # colidx: group g's list l = t*128+j -> node of (edge g*128+j, k=t)
        colidx = np.zeros((128, 128), np.int16)
        for g in range(8):
            idx_list = nsub[g * 128 + jj2, tt]
            colidx[16 * g : 16 * (g + 1), :] = idx_list.reshape(128, 16).T
        # ---- permutation routing ----
        rng = np.random.default_rng(1234 + c)
        p2 = (node_of % 128).astype(np.int64)
        for _try in range(50):
            perms = np.stack([rng.permutation(128) for _ in range(128)])
            q = perms[p_of, blk]
            loads = np.zeros((128, 128), np.int64)
            np.add.at(loads, (q, p2), 1)
            if loads.max() <= 8:
                break
        else:
            raise RuntimeError("routing failed")

        def ranks_of(key):
            order = np.argsort(key, kind="stable")
            sk = key[order]
            seg_start = np.r_[0, np.nonzero(sk[1:] != sk[:-1])[0] + 1]
            starts = np.repeat(seg_start, np.diff(np.r_[seg_start, len(sk)]))
            r = np.empty(len(sk), np.int64)
            r[order] = np.arange(len(sk)) - starts
            return r

        sq2 = ranks_of(q * 128 + p2)
        assert sq2.max() < 8
        lane = ranks_of(node_of)
        assert lane.max() < LANES, f"lane overflow {lane.max()}"

        r1 = perms.astype(np.int16)               # r1[p1, f1] = q
        r2 = np.empty((128, 128), np.int16)
        r2[q, p_of] = (sq2 * 128 + p2).astype(np.int16)
        r3 = np.full((128, 1024), -1, np.int16)
        r3[p2, sq2 * 128 + q] = ((node_of // 128) * LANES + lane).astype(np.int16)

        i16b_l = np.concatenate([colidx, r1, r2, r3, np.zeros((128, 16), np.int16)], axis=1).astype(np.int16).copy()

        # ---- fp8 incidence for the final matmul (identity edge grouping) --
        inc8 = np.zeros((N, 8, 128), np.float32)   # [node, j, m-col]
        mm = np.arange(ML)
        inc8[nsub, (mm // 128)[:, None], (mm % 128)[:, None]] = 1.0
        inc8 = inc8.reshape(64, 128, 8, 128).transpose(1, 2, 0, 3)
        inc8 = inc8.astype(ml_dtypes.float8_e4m3).reshape(128, -1).copy()

        in_maps.append(
            {
                "wenc_in": wenc_l,
                "ift_in": ift_l,
                "wattr_in": wattr_l,
                "i16b_in": i16b_l,
                "bfb_in": bfb_l,
                "f32b_in": f32b_l,
                "inc_in": inc8,
            }
        )
    return in_maps


LAST_RESULT = None


def kernel(**inputs):
    global LAST_RESULT
    from concourse import bass_utils

    if "nc" not in _CACHE:
        _CACHE["nc"] = _build_program()
    nc = _CACHE["nc"]
    in_maps = _host_prep(inputs)
    res = bass_utils.run_bass_kernel_spmd(
        nc, in_maps, core_ids=list(range(NCORES))
    )
    LAST_RESULT = res
    out = np.empty((M, 2), np.float32)
    for c in range(NCORES):
        o = res.results[c]["out"].reshape(128, 8, 2)   # [p, j, c]
        out[c * ML : (c + 1) * ML] = o.transpose(1, 0, 2).reshape(ML, 2)
    return out


# revision 13
# speedup vs baseline: 1.0916x; 1.0916x over previous
"""Trainium2 Bass kernel for nn_CLOSEgaps (hypergraph attention conv), 8 NeuronCores.

Algorithmic collapse (validated vs reference + numpy mirror):
  per-node 12-vector table:
    s_n[h] = x @ (W_conv_h @ att[h,:128]);  p[h,c] = x @ (W_conv_h @ W_out)
    wav[h] = (W_attr + b_attr/16) @ (W_conv_h @ att[h,128:])
  per-pair: e = exp(lrelu(s_n + s_e)), Z, T, qq, v (2 f32).
  zacc[n,c] = sum of v over pairs at n;  z = Dv*zacc + C
  out[m,c] = sum_{n in e_m} z[n,c]  (fp8 incidence matmul on PE)

v4: the HBM DMA-gather (16K descriptors, ~100us of Pool desc-gen) is
replaced by an all-SBUF channel-major pipeline:
  - table built TRANSPOSED ([12ch, 1024 nodes] per core) directly by PE,
    AllGathered as [128=(rank,ch), 1024], then replicated to all 8
    Q7 groups via one-hot partition-mix matmuls -> Trep [128, 8192] f32.
  - ONE GpSimd ap_gather (per-16-partition-group indices = global node
    ids) expands Trep to per-pair columns Gcol [128, 2048].
  - attention runs column-major: free dim = (k=16, edge=128) per group,
    per-edge softmax reductions are stride-128 free reduces; cross-
    channel moves are one-hot stationary matmuls (Msn/Me).
  - v is transposed back to row-major pair planes (16 PE transposes) and
    fed to the unchanged local_scatter permutation network -> zacc ->
    AllReduce -> z -> fp8 incidence matmul -> out.
"""
import sys

sys.path.insert(0, "/opt/trn_rl_repo")

import numpy as np

N = 8192
M = 8192
F_IN = 512
EMB = 256
CD = 128
H = 3
DEG = 16
NCORES = 8
NL = N // NCORES      # nodes per core
ML = M // NCORES      # edges per core
EL = ML * DEG         # pairs per core = 16384
LANES = 16            # node-slot lanes in the permuted layout

_CACHE = {}


def _build_program():
    import concourse.bass as bass
    import concourse.bacc as bacc
    import concourse.tile as tile
    from concourse import mybir

    f32 = mybir.dt.float32
    i16 = mybir.dt.int16
    bf16 = mybir.dt.bfloat16

    nc = bacc.Bacc("TRN2", target_bir_lowering=False, debug=False, num_devices=NCORES)

    # ---- per-core external inputs (host pre-laid-out) ----
    wenc_in = nc.dram_tensor("wenc_in", [128, 4 * 256], bf16, kind="ExternalInput").ap()
    ift_in = nc.dram_tensor("ift_in", [128, 4 * 1024], bf16, kind="ExternalInput").ap()
    wattr_in = nc.dram_tensor("wattr_in", [128, 2 * 1024], bf16, kind="ExternalInput").ap()
    # i16 blob: colidx [128] | r1 [128] | r2 [128] | r3 [1024]
    i16b_in = nc.dram_tensor("i16b_in", [128, 1424], i16, kind="ExternalInput").ap()
    # bf16 blob: wconv [768] | pproj [36] | brep [8*128] | msn [128] | me [128] | ident [128]
    BFW = 768 + 36 + 8 * 128 + 128
    bfb_in = nc.dram_tensor("bfb_in", [128, BFW], bf16, kind="ExternalInput").ap()
    # f32 blob: benc [2] | dv [64] | cc [2]
    f32b_in = nc.dram_tensor("f32b_in", [128, 68], f32, kind="ExternalInput").ap()
    inc_in = nc.dram_tensor("inc_in", [128, 8 * 64 * 128], mybir.dt.float8e4, kind="ExternalInput").ap()
    out_dram = nc.dram_tensor("out", [128, 16], f32, kind="ExternalOutput").ap()

    with tile.TileContext(nc) as tc:
        with (
            tc.tile_pool(name="sbuf", bufs=1) as sb,
            tc.tile_pool(name="big", bufs=1) as bigp,
            tc.tile_pool(name="psum", bufs=2, space="PSUM") as ps,
            tc.tile_pool(name="psum1", bufs=1, space="PSUM") as ps1,
            tc.tile_pool(name="dram", bufs=1, space="DRAM") as dram,
        ):
            # dummy collective: absorbs the ~15-40us first-collective
            # firmware/entry cost while the encoder runs
            dumg_in = dram.tile([16, 4], f32)
            dumg_out = dram.tile([128, 4], f32)
            nc.gpsimd.collective_compute(
                "AllGather",
                mybir.AluOpType.bypass,
                replica_groups=[list(range(NCORES))],
                ins=[dumg_in.opt()],
                outs=[dumg_out.opt()],
            )

            # ------- input loads (priority order) -------
            wenc_t = sb.tile([128, 4, 256], bf16)
            nc.sync.dma_start(out=wenc_t[:].rearrange("p a b -> p (a b)"), in_=wenc_in[:])
            ift_t = sb.tile([128, 4, 1024], bf16)
            nc.sync.dma_start(out=ift_t[:].rearrange("p a b -> p (a b)"), in_=ift_in[:])
            wattr_t = sb.tile([128, 2, 1024], bf16)
            nc.sync.dma_start(out=wattr_t[:].rearrange("p a b -> p (a b)"), in_=wattr_in[:])
            i16b_t = sb.tile([128, 1424], i16)
            nc.sync.dma_start(out=i16b_t[:], in_=i16b_in[:])
            bfb_t = sb.tile([128, BFW], bf16)
            nc.sync.dma_start(out=bfb_t[:], in_=bfb_in[:])
            f32b_t = sb.tile([128, 68], f32)
            nc.sync.dma_start(out=f32b_t[:], in_=f32b_in[:])

            colidx = i16b_t[:, 0:128]
            r1_v = i16b_t[:, 128:256]
            r2_v = i16b_t[:, 256:384]
            r3_v = i16b_t[:, 384:1408]
            zidx_v = i16b_t[:, 1408:1424]
            wconv_v = bfb_t[:, 0:768].rearrange("p (a b) -> p a b", a=3)
            pproj_v = bfb_t[:, 768:804].rearrange("p (a b) -> p a b", a=3)
            brep_v = bfb_t[:, 804:1828].rearrange("p (a b) -> p a b", a=8)
            ident_v = bfb_t[:, 1828:1956]
            benc_v = f32b_t[:, 0:2]
            dv_v = f32b_t[:, 2:66]
            cc_v = f32b_t[:, 66:68]

            # dummy ap_gather: triggers the ucode library IRAM load early so
            # the real gather doesn't stall ~57us on LOAD_LIB
            scrapG = sb.tile([128, 16], f32)
            nc.gpsimd.ap_gather(
                out_ap=scrapG[:], in_ap=f32b_t[:, 0:64], idxs_ap=zidx_v,
                channels=128, num_elems=64, d=1, num_idxs=16,
            )

            # ------- P1: xT[e, n] = relu(W_enc.T @ IF.T + b_enc), emb-major --
            xT_t = sb.tile([128, 2, 1024], bf16)
            for eh in range(2):
                for nh in range(2):
                    px = ps.tile([128, 512], f32, tag="px")
                    for kc in range(4):
                        nc.tensor.matmul(
                            out=px[:],
                            lhsT=wenc_t[:, kc, eh * 128 : (eh + 1) * 128],
                            rhs=ift_t[:, kc, nh * 512 : (nh + 1) * 512],
                            start=(kc == 0),
                            stop=(kc == 3),
                        )
                    nc.scalar.activation(
                        out=xT_t[:, eh, nh * 512 : (nh + 1) * 512],
                        in_=px[:],
                        func=mybir.ActivationFunctionType.Relu,
                        bias=benc_v[:, eh : eh + 1],
                    )

            # ------- P2: UV = W_conv.T-chunks @ P_proj  ([256(2x128), 12]) --
            uv_t = sb.tile([128, 2, 12], bf16)
            for eh in range(2):
                pu_full = ps.tile([128, 512], f32, tag="px")
                pu = pu_full[0:128, 0:12]
                for qc in range(3):
                    nc.tensor.matmul(
                        out=pu,
                        lhsT=wconv_v[:, qc, eh * 128 : (eh + 1) * 128],
                        rhs=pproj_v[:, qc, :],
                        start=(qc == 0),
                        stop=(qc == 2),
                    )
                nc.vector.tensor_copy(uv_t[:, eh, :], pu)

            # ------- P2bT: transposed table slab tsliceT[16, 1024] ----------
            # zero-padded uv stationaries so all 12 rows accumulate in one
            # PSUM tile starting at partition 0 (no partition-offset writes)
            uvz = sb.tile([128, 4, 12], bf16)
            nc.vector.memset(uvz[:], 0.0)
            for eh in range(2):
                nc.vector.tensor_copy(uvz[:, eh, 0:9], uv_t[:, eh, 0:9])
                nc.vector.tensor_copy(uvz[:, 2 + eh, 9:12], uv_t[:, eh, 9:12])
            tsl_sb = sb.tile([16, 1024], bf16)
            nc.vector.memset(tsl_sb[:], 0.0)
            for jb in range(2):
                pt16 = ps.tile([128, 512], f32, tag="px")
                for eh in range(2):
                    nc.tensor.matmul(
                        out=pt16[0:12, :],
                        lhsT=uvz[:, eh, :],
                        rhs=xT_t[:, eh, jb * 512 : (jb + 1) * 512],
                        start=(eh == 0),
                        stop=False,
                    )
                for ec in range(2):
                    nc.tensor.matmul(
                        out=pt16[0:12, :],
                        lhsT=uvz[:, 2 + ec, :],
                        rhs=wattr_t[:, ec, jb * 512 : (jb + 1) * 512],
                        start=False,
                        stop=(ec == 1),
                    )
                nc.vector.tensor_copy(tsl_sb[0:12, jb * 512 : (jb + 1) * 512], pt16[0:12, :])

            # ------- AllGather transposed table: trepg [128=(r,ch), 1024] ---
            tsliceT = dram.tile([16, 1024], bf16)
            nc.sync.dma_start(out=tsliceT[:], in_=tsl_sb[:])
            trepg_d = dram.tile([128, 1024], bf16)
            nc.gpsimd.collective_compute(
                "AllGather",
                mybir.AluOpType.bypass,
                replica_groups=[list(range(NCORES))],
                ins=[tsliceT.opt()],
                outs=[trepg_d.opt()],
            )
            trepg_bf = sb.tile([128, 1024], bf16)
            nc.sync.dma_start(out=trepg_bf[:], in_=trepg_d[:])

            # ------- Trep build: [128=(g,ch), 8192 nodes] f32 ----------------
            trep = bigp.tile([128, 8, 1024], f32)
            for r in range(8):
                for jb in range(2):
                    pm = ps.tile([128, 512], f32, tag="px")
                    nc.tensor.matmul(
                        out=pm[:],
                        lhsT=brep_v[:, r, :],
                        rhs=trepg_bf[:, jb * 512 : (jb + 1) * 512],
                        start=True,
                        stop=True,
                    )
                    if jb == 0:
                        nc.vector.tensor_copy(trep[:, r, 0:512], pm[:])
                    else:
                        nc.scalar.activation(
                            out=trep[:, r, 512:1024], in_=pm[:],
                            func=mybir.ActivationFunctionType.Copy,
                        )

            # ------- THE gather: Gcol[16g+ch, t*128+j] = table[ch, node] ----
            Gcol = sb.tile([128, 2048], f32)
            nc.gpsimd.ap_gather(
                out_ap=Gcol[:],
                in_ap=trep[:].rearrange("p a b -> p (a b)"),
                idxs_ap=colidx,
                channels=128,
                num_elems=N,
                d=1,
                num_idxs=2048,
            )

            # ------- non-critical loads ----
            incs_t = bigp.tile([128, 8, 64, 128], mybir.dt.float8e4)
            nc.sync.dma_start(out=incs_t[:].rearrange("p a b m -> p (a b m)"), in_=inc_in[:])

            # ------- row-major conversion: 16 PE transposes ----------------
            Gbf = sb.tile([128, 2048], bf16)
            nc.scalar.activation(out=Gbf[:], in_=Gcol[:], func=mybir.ActivationFunctionType.Copy)
            Grow = sb.tile([128, 16, 128], bf16)
            for t in range(16):
                ptt = ps.tile([128, 128], bf16, tag="pt")
                nc.tensor.transpose(ptt[:], Gbf[:, t * 128 : (t + 1) * 128], ident_v)
                nc.vector.tensor_copy(Grow[:, t, :], ptt[:])
            # Grow[p, t, 16g+ch]: pair (p, blk=16g+t); ch: 0:3 sn, 3:9 p(h,c), 9:12 wav

            # ------- attention, row-major --------------------------------
            gv = Grow[:].rearrange("p t (g s) -> p t g s", s=16)
            gvT = Grow[:].rearrange("p t (g s) -> p g s t", s=16)
            se8 = sb.tile([128, 8, 3], f32)
            nc.vector.reduce_sum(
                out=se8[:][:, :, :, None], in_=gvT[:, :, 9:12, :],
                axis=mybir.AxisListType.X,
            )
            lg = sb.tile([128, 16, 8, 3], f32)
            nc.vector.tensor_tensor(
                out=lg[:], in0=gv[:, :, :, 0:3],
                in1=se8[:][:, None, :, :].to_broadcast([128, 16, 8, 3]),
                op=mybir.AluOpType.add,
            )
            lgs = sb.tile([128, 16, 8, 3], f32)
            nc.vector.tensor_scalar_mul(lgs[:], lg[:], 0.2)
            nc.vector.tensor_tensor(out=lg[:], in0=lg[:], in1=lgs[:], op=mybir.AluOpType.max)
            eev = sb.tile([128, 16, 8, 3], bf16)
            nc.scalar.activation(out=eev[:], in_=lg[:], func=mybir.ActivationFunctionType.Exp)

            zz8 = sb.tile([128, 8, 3], f32)
            nc.vector.reduce_sum(
                out=zz8[:][:, :, :, None],
                in_=eev[:].rearrange("p t g h -> p g h t"),
                axis=mybir.AxisListType.X,
            )
            nc.vector.tensor_scalar_add(zz8[:], zz8[:], 1e-16)
            zr8 = sb.tile([128, 8, 3], f32)
            nc.vector.reciprocal(zr8[:], zz8[:])
            nc.vector.tensor_tensor(out=zr8[:], in0=zr8[:], in1=zr8[:], op=mybir.AluOpType.mult)
            nc.vector.tensor_scalar_mul(zr8[:], zr8[:], 1.0 / DEG)

            tqv = sb.tile([128, 16, 8, 3, 2], bf16)
            nc.vector.tensor_tensor(
                out=tqv[:],
                in0=gv[:, :, :, 3:9].rearrange("p t g (h c) -> p t g h c", c=2),
                in1=eev[:][:, :, :, :, None].to_broadcast([128, 16, 8, 3, 2]),
                op=mybir.AluOpType.mult,
            )
            tt8 = sb.tile([128, 8, 3, 2], f32)
            nc.vector.reduce_sum(
                out=tt8[:][:, :, :, :, None],
                in_=tqv[:].rearrange("p t g h c -> p g h c t"),
                axis=mybir.AxisListType.X,
            )
            qq8 = sb.tile([128, 8, 3, 2], f32)
            nc.vector.tensor_tensor(
                out=qq8[:], in0=tt8[:],
                in1=zr8[:][:, :, :, None].to_broadcast([128, 8, 3, 2]),
                op=mybir.AluOpType.mult,
            )
            vh = sb.tile([128, 16, 8, 3, 2], f32)
            nc.vector.tensor_tensor(
                out=vh[:],
                in0=eev[:][:, :, :, :, None].to_broadcast([128, 16, 8, 3, 2]),
                in1=qq8[:][:, None, :, :, :].to_broadcast([128, 16, 8, 3, 2]),
                op=mybir.AluOpType.mult,
            )
            vtv = sb.tile([128, 16, 8, 2], f32)
            nc.vector.reduce_sum(
                out=vtv[:][:, :, :, :, None],
                in_=vh[:].rearrange("p t g h c -> p t g c h"),
                axis=mybir.AxisListType.X,
            )
            V0 = sb.tile([128, 128], bf16)
            V1 = sb.tile([128, 128], bf16)
            nc.vector.tensor_copy(
                V0[:].rearrange("p (g t2) -> p g t2", t2=16),
                vtv[:, :, :, 0].rearrange("p t g -> p g t"),
            )
            nc.vector.tensor_copy(
                V1[:].rearrange("p (g t2) -> p g t2", t2=16),
                vtv[:, :, :, 1].rearrange("p t g -> p g t"),
            )

            # ------- permutation network: R1 -> T -> R2 -> T -> R3 -------
            S1a = sb.tile([128, 128], bf16)
            S1b = sb.tile([128, 128], bf16)
            S1Ta = sb.tile([128, 128], bf16)
            S1Tb = sb.tile([128, 128], bf16)
            S2a = sb.tile([128, 1024], bf16)
            S2b = sb.tile([128, 1024], bf16)
            S2Ta = sb.tile([128, 1024], bf16)
            S2Tb = sb.tile([128, 1024], bf16)
            S3a = sb.tile([128, 1024], bf16)
            S3b = sb.tile([128, 1024], bf16)
            S1 = [S1a, S1b]
            S1T = [S1Ta, S1Tb]
            S2 = [S2a, S2b]
            S2T = [S2Ta, S2Tb]
            S3 = [S3a, S3b]
            for c, V in enumerate([V0, V1]):
                nc.gpsimd.local_scatter(
                    out_ap=S1[c][:], data_ap=V[:], idxs_ap=r1_v,
                    channels=128, num_elems=128, num_idxs=128,
                )
            for c in range(2):
                pt = ps1.tile([128, 128], bf16, tag="pt")
                nc.tensor.transpose(pt[:], S1[c][:], ident_v)
                nc.vector.tensor_copy(S1T[c][:], pt[:])
            for c in range(2):
                nc.gpsimd.local_scatter(
                    out_ap=S2[c][:], data_ap=S1T[c][:], idxs_ap=r2_v,
                    channels=128, num_elems=1024, num_idxs=128,
                )
            for c in range(2):
                for sq in range(8):
                    pt = ps1.tile([128, 128], bf16, tag="pt")
                    nc.tensor.transpose(
                        pt[:], S2[c][:, 128 * sq : 128 * (sq + 1)], ident_v
                    )
                    nc.vector.tensor_copy(S2T[c][:, 128 * sq : 128 * (sq + 1)], pt[:])
            for c in range(2):
                nc.gpsimd.local_scatter(
                    out_ap=S3[c][:], data_ap=S2T[c][:], idxs_ap=r3_v,
                    channels=128, num_elems=1024, num_idxs=1024,
                )

            # ------- lane reduce: zacc[p, nr, c], node = nr*128 + p -------
            zacc = sb.tile([128, 64, 2], f32)
            for c in range(2):
                nc.vector.reduce_sum(
                    out=zacc[:, :, c][:, :, None],
                    in_=S3[c][:].rearrange("p (nr l) -> p nr l", l=LANES),
                    axis=mybir.AxisListType.X,
                )

            # ------- AllReduce zacc -------
            ar_in = dram.tile([128, 128], f32)
            nc.sync.dma_start(out=ar_in[:], in_=zacc[:].rearrange("p a b -> p (a b)"))
            ar_out = dram.tile([128, 128], f32)
            nc.gpsimd.collective_compute(
                "AllReduce",
                mybir.AluOpType.add,
                replica_groups=[list(range(NCORES))],
                ins=[ar_in.opt()],
                outs=[ar_out.opt()],
            )
            zred = sb.tile([128, 64, 2], f32)
            nc.sync.dma_start(out=zred[:].rearrange("p a b -> p (a b)"), in_=ar_out[:])

            # ------- z = Dv*zacc + C; bf16 hi/lo split -------
            nc.vector.tensor_tensor(
                out=zred[:],
                in0=zred[:],
                in1=dv_v[:, :, None].to_broadcast([128, 64, 2]),
                op=mybir.AluOpType.mult,
            )
            nc.vector.tensor_tensor(
                out=zred[:],
                in0=zred[:],
                in1=cc_v[:, None, :].to_broadcast([128, 64, 2]),
                op=mybir.AluOpType.add,
            )
            zz4 = sb.tile([128, 64, 4], bf16)
            nc.vector.tensor_copy(zz4[:, :, 0:2], zred[:])
            zhi32 = sb.tile([128, 64, 2], f32)
            nc.vector.tensor_copy(zhi32[:], zz4[:, :, 0:2])
            nc.vector.tensor_tensor(
                out=zhi32[:], in0=zred[:], in1=zhi32[:], op=mybir.AluOpType.subtract
            )
            nc.vector.tensor_copy(zz4[:, :, 2:4], zhi32[:])

            # ------- final: out[p_e, c] = sum_n inc[n, edge] * z[n] via PE --
            out_t = sb.tile([128, 8, 2], f32)
            for j in range(8):
                po_full = ps1.tile([128, 128], f32, tag="seps")
                po = po_full[0:128, 0:4]
                for nck in range(64):
                    nc.tensor.matmul(
                        out=po,
                        lhsT=incs_t[:, j, nck, :],
                        rhs=zz4[:, nck, :],
                        start=(nck == 0),
                        stop=(nck == 63),
                    )
                nc.vector.tensor_copy(out_t[:, j, :], po_full[:, 0:2])
                nc.vector.tensor_tensor(
                    out=out_t[:, j, :], in0=out_t[:, j, :], in1=po_full[:, 2:4],
                    op=mybir.AluOpType.add,
                )
            nc.sync.dma_start(
                out=out_dram[:], in_=out_t[:].rearrange("p a b -> p (a b)")
            )

    nc.compile()
    return nc


def _host_prep(inputs):
    """Build per-core in_maps from full inputs."""
    import ml_dtypes

    bf = ml_dtypes.bfloat16
    IF = np.asarray(inputs["input_features"], np.float32)
    node_idx = np.asarray(inputs["node_idx"])
    W_enc = np.asarray(inputs["W_enc"], np.float32)
    b_enc = np.asarray(inputs["b_enc"], np.float32)
    W_attr = np.asarray(inputs["W_attr"], np.float32)
    b_attr = np.asarray(inputs["b_attr"], np.float32)
    W_conv = np.asarray(inputs["W_conv"], np.float32)
    att = np.asarray(inputs["att"], np.float32)
    b_conv = np.asarray(inputs["b_conv"], np.float32)
    W_out = np.asarray(inputs["W_out"], np.float32)
    b_out = np.asarray(inputs["b_out"], np.float32)

    nodes16 = node_idx.reshape(M, DEG).astype(np.int64)

    # weight prep
    P_proj = np.zeros((H * CD, 12), np.float32)
    for h in range(H):
        P_proj[h * CD : (h + 1) * CD, h] = att[h, :CD]
        for cc in range(2):
            P_proj[h * CD : (h + 1) * CD, 3 + h * 2 + cc] = W_out[h * CD : (h + 1) * CD, cc]
        P_proj[h * CD : (h + 1) * CD, 9 + h] = att[h, CD:]

    deg_n = np.bincount(node_idx, minlength=N)
    Dv = np.where(deg_n > 0, 1.0 / np.maximum(deg_n, 1), 0.0).astype(np.float32)
    C = (b_conv @ W_out + b_out / DEG).astype(np.float32)

    wenc_l = W_enc.reshape(4, 128, EMB).transpose(1, 0, 2).reshape(128, -1).astype(bf).copy()
    benc_l = b_enc.reshape(2, 128).T.astype(np.float32)
    wconv_l = W_conv.T.reshape(3, 128, EMB).transpose(1, 0, 2).reshape(128, -1)
    pproj_l = P_proj.reshape(3, 128, 12).transpose(1, 0, 2).reshape(128, -1)
    cc_l = np.tile(C[None, :], (128, 1))
    dv_l = Dv.reshape(64, 128).T

    # one-hot partition-mix matrices
    brep = np.zeros((128, 8, 128), np.float32)
    for r_ in range(8):
        for g in range(8):
            for ch in range(12):
                brep[16 * r_ + ch, r_, 16 * g + ch] = 1.0
    ident_np = np.eye(128, dtype=np.float32)
    bfb_l = np.concatenate(
        [wconv_l, pproj_l, brep.reshape(128, -1), ident_np], axis=1
    ).astype(bf).copy()
    f32b_l = np.concatenate([benc_l, dv_l, cc_l], axis=1).astype(np.float32).copy()

    # pair order: i = blk*128 + p, blk = 16g+t; edge = g*128+p, k=t
    i = np.arange(EL)
    p_of = i % 128
    blk = i // 128
    jj = blk // 16
    k_of = blk % 16

    ll = np.arange(2048)
    jj2 = ll % 128
    tt = ll // 128

    in_maps = []
    for c in range(NCORES):
        nsl = slice(c * NL, (c + 1) * NL)
        esl = slice(c * ML, (c + 1) * ML)
        ift_l = (
            IF[nsl].T.reshape(4, 128, 1024).transpose(1, 0, 2).reshape(128, -1).astype(bf).copy()
        )
        wattr_l = (
            (W_attr[nsl] + b_attr[None, :] / DEG)
            .T.reshape(2, 128, 1024)
            .transpose(1, 0, 2)
            .reshape(128, -1)
            .astype(bf)
            .copy()
        )
        nsub = nodes16[esl]                       # [1024, 16]
        node_of = nsub[jj * 128 + p_of, k_of]     # node id per pair i

        # colidx: group g's list l = t*128+j -> node of (edge g*128+j, k=t)
        colidx = np.zeros((128, 128), np.int16)
        for g in range(8):
            idx_list = nsub[g * 128 + jj2, tt]
            colidx[16 * g : 16 * (g + 1), :] = idx_list.reshape(128, 16).T
        # ---- permutation routing ----
        rng = np.random.default_rng(1234 + c)
        p2 = (node_of % 128).astype(np.int64)
        for _try in range(50):
            perms = np.stack([rng.permutation(128) for _ in range(128)])
            q = perms[p_of, blk]
            loads = np.zeros((128, 128), np.int64)
            np.add.at(loads, (q, p2), 1)
            if loads.max() <= 8:
                break
        else:
            raise RuntimeError("routing failed")

        def ranks_of(key):
            order = np.argsort(key, kind="stable")
            sk = key[order]
            seg_start = np.r_[0, np.nonzero(sk[1:] != sk[:-1])[0] + 1]
            starts = np.repeat(seg_start, np.diff(np.r_[seg_start, len(sk)]))
            r = np.empty(len(sk), np.int64)
            r[order] = np.arange(len(sk)) - starts
            return r

        sq2 = ranks_of(q * 128 + p2)
        assert sq2.max() < 8
        lane = ranks_of(node_of)
        assert lane.max() < LANES, f"lane overflow {lane.max()}"

        r1 = perms.astype(np.int16)               # r1[p1, f1] = q
        r2 = np.empty((128, 128), np.int16)
        r2[q, p_of] = (sq2 * 128 + p2).astype(np.int16)
        r3 = np.full((128, 1024), -1, np.int16)
        r3[p2, sq2 * 128 + q] = ((node_of // 128) * LANES + lane).astype(np.int16)

        i16b_l = np.concatenate([colidx, r1, r2, r3, np.zeros((128, 16), np.int16)], axis=1).astype(np.int16).copy()

        # ---- fp8 incidence for the final matmul (identity edge grouping) --
        inc8 = np.zeros((N, 8, 128), np.float32)   # [node, j, m-col]
        mm = np.arange(ML)
        inc8[nsub, (mm // 128)[:, None], (mm % 128)[:, None]] = 1.0
        inc8 = inc8.reshape(64, 128, 8, 128).transpose(1, 2, 0, 3)
        inc8 = inc8.astype(ml_dtypes.float8_e4m3).reshape(128, -1).copy()

        in_maps.append(
            {
                "wenc_in": wenc_l,
                "ift_in": ift_l,
                "wattr_in": wattr_l,
                "i16b_in": i16b_l,
                "bfb_in": bfb_l,
                "f32b_in": f32b_l,
                "inc_in": inc8,
            }
        )
    return in_maps


LAST_RESULT = None


def kernel(**inputs):
    global LAST_RESULT
    from concourse import bass_utils

    if "nc" not in _CACHE:
        _CACHE["nc"] = _build_program()
    nc = _CACHE["nc"]
    in_maps = _host_prep(inputs)
    res = bass_utils.run_bass_kernel_spmd(
        nc, in_maps, core_ids=list(range(NCORES))
    )
    LAST_RESULT = res
    out = np.empty((M, 2), np.float32)
    for c in range(NCORES):
        o = res.results[c]["out"].reshape(128, 8, 2)   # [p, j, c]
        out[c * ML : (c + 1) * ML] = o.transpose(1, 0, 2).reshape(ML, 2)
    return out
